# revision 1
# baseline (speedup 1.0000x reference)
"""Trainium2 Bass kernel for nn_DiffForest (soft decision forest forward).

Math: per tree t, z = x @ w_d[t]; p = sigmoid(z); leaf path probs are products
of 8 factors p/(1-p) down a depth-8 tree; output = sum_t leaf_prob @ softmax(w_l[t]) / 10.

Kernel formulation (all on device except small weight prep):
  - The 512 "leaves" come in identical pairs -> fold to 256 paths; fold the
    pair-sum + 1/n_trees into the leaf weight matrix w2 (host, exact).
  - Path products move to log space:  -log P[q] = sum_path softplus(-z) + sum_{branch=1} z
    which is a matmul with a constant 0/1 matrix S [512, 256]:
        A = S^T @ [softplus(-z); z],   leaf_prob^T = exp(-A)   ([256 paths, batch])
    softplus(-z) = ln(1 + exp(-z)) via the Exp/Ln activation tables (one table set).
  - This leaves three matmul stages (decision matmul in bf16, S-matmul in fp32r,
    leaf matmul in bf16) with the contraction dim on partitions throughout; no
    on-device transposes are needed because the S-matmul naturally produces
    leaf-major layout.
  - Sharding: data-parallel over batch; each of the 8 cores takes 2048 rows of x,
    weights replicated, no collectives.
"""

import numpy as np
import ml_dtypes

import concourse.bacc as bacc
import concourse.mybir as mybir
import concourse.tile as tile
from concourse.tile import add_dep_helper
from concourse.bass_utils import run_bass_kernel_spmd

N_CORES = 8
BATCH = 16384
B_LOC = BATCH // N_CORES        # 2048 rows per core
IN_DIM = 2048
N_TREES = 10
ND_PAD = 256                    # decision nodes padded 255 -> 256
NQ = 256                        # folded path (leaf) count
CLASSES = 1000
CHUNK = 512                     # batch columns processed per chunk
KI = IN_DIM // 128              # 16 contraction tiles for the decision matmul

BF16 = mybir.dt.bfloat16
F32 = mybir.dt.float32
F32R = mybir.dt.float32r
F16 = mybir.dt.float16
AF = mybir.ActivationFunctionType

import os

_CACHE = {}



def _build(b_loc=B_LOC, n_trees=N_TREES):
    n_chunks = b_loc // CHUNK
    nc = bacc.Bacc("TRN2", target_bir_lowering=False)
    xt = nc.dram_tensor("xt", (IN_DIM, b_loc), BF16, kind="ExternalInput")
    wd = nc.dram_tensor("wd", (n_trees, IN_DIM, ND_PAD), BF16, kind="ExternalInput")
    smat = nc.dram_tensor("smat", (512, NQ), F32R, kind="ExternalInput")
    w2 = nc.dram_tensor("w2", (n_trees, NQ, CLASSES), BF16, kind="ExternalInput")
    out = nc.dram_tensor("out", (b_loc, CLASSES), F32, kind="ExternalOutput")

    with tile.TileContext(nc) as tc:
        with (
            tc.tile_pool(name="const", bufs=1) as constp,
            tc.tile_pool(name="sb", bufs=2) as sb,
            tc.tile_pool(name="wdp", bufs=3) as wdp,
            tc.tile_pool(name="ep", bufs=6) as ep,
            tc.tile_pool(name="gp", bufs=6) as gp,
            tc.tile_pool(name="outp", bufs=2) as outp,
            tc.tile_pool(name="lptp", bufs=1) as lptp,
            tc.tile_pool(name="pz", bufs=2, space="PSUM") as pzp,
            tc.tile_pool(name="plp", bufs=2, space="PSUM") as plpp,
            tc.tile_pool(name="po", bufs=2, space="PSUM") as pop,
        ):
            smat_sb = constp.tile([128, 4, NQ], F32R)
            w2_sb = constp.tile([128, n_trees, 2, CLASSES], BF16)

            GROUP = 5
            first_mm = [None]
            started = False

            def emit_mm2(ci, lpT):
                c0 = ci * CHUNK
                for s in range(CHUNK // 128):
                    po = pop.tile([128, 1024], F32, tag="po")
                    n_acc = n_trees * 2
                    i = 0
                    for t in range(n_trees):
                        for lt in range(2):
                            first = i == 0
                            last = i == n_acc - 1
                            lhsT = lpT[:, t, lt, s * 128 : (s + 1) * 128]
                            nc.tensor.matmul(
                                po[:, 0:500], lhsT, w2_sb[:, t, lt, 0:500],
                                start=first, stop=last,
                            )
                            nc.tensor.matmul(
                                po[:, 512:1012], lhsT, w2_sb[:, t, lt, 500:1000],
                                start=first, stop=last,
                            )
                            i += 1
                    osb = outp.tile([128, CLASSES], F32, tag="osb")
                    nc.vector.tensor_copy(osb[:, 0:500], po[:, 0:500])
                    nc.vector.tensor_copy(osb[:, 500:1000], po[:, 512:1012])
                    nc.sync.dma_start(
                        out[c0 + s * 128 : c0 + (s + 1) * 128, :], osb[:, :]
                    )

            for ci in range(n_chunks):
                c0 = ci * CHUNK
                xt_pieces = []
                for kq in range(4):
                    xp = sb.tile([128, 4, CHUNK], BF16, tag=f"xt{kq}")
                    xdma = nc.sync.dma_start(
                        xp[:, :, :],
                        xt[
                            4 * kq * 128 : 4 * (kq + 1) * 128, c0 : c0 + CHUNK
                        ].rearrange("(k p) n -> p k n", p=128),
                    )
                    xt_pieces.append(xp)
                lpT = lptp.tile([128, n_trees, 2, CHUNK], BF16, tag="lpT")
                for t0 in range(0, n_trees, GROUP):
                    group = list(range(t0, min(t0 + GROUP, n_trees)))
                    group_G = {}
                    group_E = {}
                    last_exp = None
                    for t in group:
                        wd_pieces = None
                        wd_sb = None
                        if ci == 0 and t == 0:
                            # first tree: 4 piece tiles -> the first matmul
                            # only waits on one 256KB piece + one xt piece
                            wd_pieces = []
                            for kq in range(4):
                                wp = constp.tile(
                                    [128, 4, ND_PAD], BF16, tag=f"wd0p{kq}"
                                )
                                wdma = nc.sync.dma_start(
                                    wp[:, :, :],
                                    wd[
                                        t, 4 * kq * 128 : 4 * (kq + 1) * 128, :
                                    ].rearrange("(k p) d -> p k d", p=128),
                                )
                                wd_pieces.append(wp)
                        else:
                            wd_sb = wdp.tile([128, KI, ND_PAD], BF16, tag="wd")
                            wd_dma = nc.sync.dma_start(
                                wd_sb[:, :, :],
                                wd[t, :, :].rearrange("(k p) d -> p k d", p=128),
                            )
                            if ci == 0 and t in (1, 2):
                                add_dep_helper(
                                    wd_dma.ins, first_mm[0].ins, sync=True,
                                    reason="startup: critical pieces first",
                                )
                        G = gp.tile([128, 4, CHUNK], F32R, tag="G")
                        E = ep.tile([128, 2, CHUNK], F16, tag="E")
                        group_G[t] = G
                        group_E[t] = E
                        for dt_ in range(2):
                            psz = pzp.tile([128, CHUNK], F32, tag="psz")
                            for k in range(KI):
                                if wd_sb is None:
                                    lhsT = wd_pieces[k // 4][
                                        :, k % 4, dt_ * 128 : (dt_ + 1) * 128
                                    ]
                                else:
                                    lhsT = wd_sb[:, k, dt_ * 128 : (dt_ + 1) * 128]
                                mm = nc.tensor.matmul(
                                    psz[:, :],
                                    lhsT,
                                    xt_pieces[k // 4][:, k % 4, :],
                                    start=(k == 0),
                                    stop=(k == KI - 1),
                                )
                                if first_mm[0] is None:
                                    first_mm[0] = mm
                            # Exp heads the ACT critical chain; CAST second
                            last_exp = nc.scalar.activation(
                                E[:, dt_, :], psz[:, :], AF.Exp, scale=-1.0
                            )
                            nc.vector.tensor_copy(G[:, 2 + dt_, :], psz[:, :])
                    # softplus(-z) = ln(exp(-z)+1); gate Lns on the group's
                    # last Exp to batch ACT table sets
                    for t in group:
                        for dt_ in range(2):
                            ln = nc.scalar.activation(
                                group_G[t][:, dt_, :],
                                group_E[t][:, dt_, :],
                                AF.Ln,
                                bias=1.0,
                            )
                            add_dep_helper(
                                ln.ins, last_exp.ins, sync=False,
                                reason="batch ACT Ln block after Exp block",
                            )
                    if not started:
                        nc.sync.dma_start(
                            smat_sb[:, :, :],
                            smat[:, :].rearrange("(k p) q -> p k q", p=128),
                        )
                        for t in range(n_trees):
                            nc.sync.dma_start(
                                w2_sb[:, t, :, :],
                                w2[t, :, :].rearrange("(l p) c -> p l c", p=128),
                            )
                        started = True
                    for t in group:
                        for lt in range(2):
                            plp = plpp.tile([128, CHUNK], F32, tag="plp")
                            for k in range(4):
                                nc.tensor.matmul(
                                    plp[:, :],
                                    smat_sb[:, k, lt * 128 : (lt + 1) * 128],
                                    group_G[t][:, k, :],
                                    start=(k == 0),
                                    stop=(k == 3),
                                )
                            nc.scalar.activation(
                                lpT[:, t, lt, :], plp[:, :], AF.Exp, scale=-1.0
                            )
                emit_mm2(ci, lpT)
    nc.compile()
    return nc


def _smat_np():
    S = np.zeros((512, NQ), np.float32)
    q = np.arange(NQ)
    for n in range(8):
        node = (2**n - 1) + (q >> (8 - n))
        branch = (q >> (7 - n)) & 1
        S[node, q] += 1.0
        S[256 + node, q] += branch.astype(np.float32)
    return S


def _prep_weights(w_d, w_l, n_trees=N_TREES):
    bf16 = ml_dtypes.bfloat16
    w_l = np.asarray(w_l, dtype=np.float32)
    m = w_l.max(axis=-1, keepdims=True)
    e = np.exp(w_l - m, dtype=np.float32)
    sm = e / e.sum(axis=-1, keepdims=True)
    w2 = ((sm[:, 0::2, :] + sm[:, 1::2, :]) * np.float32(1.0 / n_trees)).astype(bf16)
    wd_p = np.zeros((n_trees, IN_DIM, ND_PAD), np.float32)
    wd_p[:, :, : w_d.shape[2]] = w_d
    return wd_p.astype(bf16), _smat_np(), w2


last_bass_results = None


def kernel(x, w_d, w_l):
    global last_bass_results
    x = np.asarray(x)
    wd_bf, S, w2 = _prep_weights(np.asarray(w_d), np.asarray(w_l))
    x_bf = x.astype(ml_dtypes.bfloat16)
    in_maps = []
    for c in range(N_CORES):
        xt = np.ascontiguousarray(x_bf[c * B_LOC : (c + 1) * B_LOC, :].T)
        in_maps.append({"xt": xt, "wd": wd_bf, "smat": S, "w2": w2})
    if "nc" not in _CACHE:
        _CACHE["nc"] = _build()
    res = run_bass_kernel_spmd(_CACHE["nc"], in_maps, core_ids=list(range(N_CORES)))
    last_bass_results = res
    return np.concatenate([res.results[c]["out"] for c in range(N_CORES)], axis=0)



# revision 2
# speedup vs baseline: 1.2310x; 1.2310x over previous
"""Trainium2 Bass kernel for nn_DiffForest (soft decision forest forward).

Math: per tree t, z = x @ w_d[t]; p = sigmoid(z); leaf path probs are products
of 8 factors p/(1-p) down a depth-8 tree; output = sum_t leaf_prob @ softmax(w_l[t]) / 10.

Kernel formulation (all on device except small weight prep):
  - The 512 "leaves" come in identical pairs -> fold to 256 paths; fold the
    pair-sum + 1/n_trees into the leaf weight matrix w2 (host, exact).
  - Path products move to log space:  -log P[q] = sum_path softplus(-z) + sum_{branch=1} z
    which is a matmul with a constant matrix S [512, 256]:
        A = S^T @ [softplus(-z); z],   leaf_prob^T = exp(-A)   ([256 paths, batch])
    softplus(-z) = ln(1 + exp(-z)) via the Exp/Ln activation tables.
  - Decision matmul runs in fp8e4 (x scaled by 16, w_d by 64) with
    MatmulPerfMode.DoubleRow: two 128-deep k-tiles per pass, 2x PE throughput.
    The 1/1024 descale folds into the Exp activation scale and the z-half of S.
  - S-matmul in fp32r, leaf matmul in bf16 (fp8 there costs ~1.2e-2 rel err).
  - Sharding: data-parallel over batch; each of the 8 cores takes 2048 rows of x,
    weights replicated, no collectives.
"""

import numpy as np
import ml_dtypes

import concourse.bacc as bacc
import concourse.mybir as mybir
import concourse.tile as tile
from concourse.tile import add_dep_helper
from concourse.bass_utils import run_bass_kernel_spmd

N_CORES = 8
BATCH = 16384
B_LOC = BATCH // N_CORES        # 2048 rows per core
IN_DIM = 2048
N_TREES = 10
ND_PAD = 256                    # decision nodes padded 255 -> 256
NQ = 256                        # folded path (leaf) count
CLASSES = 1000
CHUNK = 512                     # batch columns processed per chunk
KI = IN_DIM // 128              # 16 contraction tiles for the decision matmul

BF16 = mybir.dt.bfloat16
F32 = mybir.dt.float32
F32R = mybir.dt.float32r
F16 = mybir.dt.float16
F8 = mybir.dt.float8e4
AF = mybir.ActivationFunctionType
DR = mybir.MatmulPerfMode.DoubleRow

X_SCALE = 16.0                  # x -> fp8
WD_SCALE = 64.0                 # w_d -> fp8
Z_DESCALE = 1.0 / (X_SCALE * WD_SCALE)   # psum holds 1024*z

_CACHE = {}


def _build(b_loc=B_LOC, n_trees=N_TREES):
    n_chunks = b_loc // CHUNK
    nc = bacc.Bacc("TRN2", target_bir_lowering=False)
    xt = nc.dram_tensor("xt", (IN_DIM, b_loc), F8, kind="ExternalInput")
    wd = nc.dram_tensor("wd", (n_trees, IN_DIM, ND_PAD), F8, kind="ExternalInput")
    smat = nc.dram_tensor("smat", (512, NQ), F32R, kind="ExternalInput")
    w2 = nc.dram_tensor("w2", (n_trees, NQ, CLASSES), BF16, kind="ExternalInput")
    out = nc.dram_tensor("out", (b_loc, CLASSES), F32, kind="ExternalOutput")

    with tile.TileContext(nc) as tc:
        with (
            tc.tile_pool(name="const", bufs=1) as constp,
            tc.tile_pool(name="sb", bufs=2) as sb,
            tc.tile_pool(name="ep", bufs=6) as ep,
            tc.tile_pool(name="gp", bufs=5) as gp,
            tc.tile_pool(name="outp", bufs=2) as outp,
            tc.tile_pool(name="lptp", bufs=1) as lptp,
            tc.tile_pool(name="pz", bufs=2, space="PSUM") as pzp,
            tc.tile_pool(name="plp", bufs=2, space="PSUM") as plpp,
            tc.tile_pool(name="po", bufs=2, space="PSUM") as pop,
        ):
            smat_sb = constp.tile([128, 4, NQ], F32R)
            w2_sb = constp.tile([128, n_trees, 2, CLASSES], BF16)

            # resident decision weights: tree 0 as 4 pieces (so the first
            # matmul only waits on a 128KB piece), trees 1..9 whole.
            wd0_pieces = []
            for kq in range(4):
                wp = constp.tile([128, 4, ND_PAD], F8, tag=f"wd0p{kq}")
                nc.sync.dma_start(
                    wp[:, :, :],
                    wd[0, 4 * kq * 128 : 4 * (kq + 1) * 128, :].rearrange(
                        "(k p) d -> p k d", p=128
                    ),
                )
                wd0_pieces.append(wp)
            wd_tiles = [None] * n_trees
            wd_dmas = []
            for t in range(1, n_trees):
                wt = constp.tile([128, KI, ND_PAD], F8, tag=f"wd{t}")
                dma = nc.sync.dma_start(
                    wt[:, :, :],
                    wd[t, :, :].rearrange("(k p) d -> p k d", p=128),
                )
                wd_tiles[t] = wt
                wd_dmas.append(dma)

            GROUP = 5
            first_mm = [None]
            started = False

            def emit_mm2(ci, lpT):
                c0 = ci * CHUNK
                for s in range(CHUNK // 128):
                    po = pop.tile([128, 1024], F32, tag="po")
                    n_acc = n_trees * 2
                    i = 0
                    for t in range(n_trees):
                        for lt in range(2):
                            first = i == 0
                            last = i == n_acc - 1
                            lhsT = lpT[:, t, lt, s * 128 : (s + 1) * 128]
                            nc.tensor.matmul(
                                po[:, 0:500], lhsT, w2_sb[:, t, lt, 0:500],
                                start=first, stop=last,
                            )
                            nc.tensor.matmul(
                                po[:, 512:1012], lhsT, w2_sb[:, t, lt, 500:1000],
                                start=first, stop=last,
                            )
                            i += 1
                    osb = outp.tile([128, CLASSES], F32, tag="osb")
                    nc.vector.tensor_copy(osb[:, 0:500], po[:, 0:500])
                    nc.vector.tensor_copy(osb[:, 500:1000], po[:, 512:1012])
                    nc.sync.dma_start(
                        out[c0 + s * 128 : c0 + (s + 1) * 128, :], osb[:, :]
                    )

            for ci in range(n_chunks):
                c0 = ci * CHUNK
                xt_pieces = []
                for kq in range(4):
                    xp = sb.tile([128, 4, CHUNK], F8, tag=f"xt{kq}")
                    nc.sync.dma_start(
                        xp[:, :, :],
                        xt[
                            4 * kq * 128 : 4 * (kq + 1) * 128, c0 : c0 + CHUNK
                        ].rearrange("(k p) n -> p k n", p=128),
                    )
                    xt_pieces.append(xp)
                lpT = lptp.tile([128, n_trees, 2, CHUNK], BF16, tag="lpT")
                for t0 in range(0, n_trees, GROUP):
                    group = list(range(t0, min(t0 + GROUP, n_trees)))
                    group_G = {}
                    group_E = {}
                    last_exp = None
                    for t in group:
                        G = gp.tile([128, 4, CHUNK], F32R, tag="G")
                        E = ep.tile([128, 2, CHUNK], F16, tag="E")
                        group_G[t] = G
                        group_E[t] = E
                        for dt_ in range(2):
                            psz = pzp.tile([128, CHUNK], F32, tag="psz")
                            # DoubleRow: 8 passes over k-tile pairs
                            for j in range(KI // 2):
                                if t == 0:
                                    lhsT = wd0_pieces[j // 2][
                                        :,
                                        2 * (j % 2) : 2 * (j % 2) + 2,
                                        dt_ * 128 : (dt_ + 1) * 128,
                                    ]
                                else:
                                    lhsT = wd_tiles[t][
                                        :, 2 * j : 2 * j + 2,
                                        dt_ * 128 : (dt_ + 1) * 128,
                                    ]
                                mm = nc.tensor.matmul(
                                    psz[:, :],
                                    lhsT,
                                    xt_pieces[j // 2][
                                        :, 2 * (j % 2) : 2 * (j % 2) + 2, :
                                    ],
                                    start=(j == 0),
                                    stop=(j == KI // 2 - 1),
                                    perf_mode=DR,
                                )
                                if first_mm[0] is None:
                                    first_mm[0] = mm
                            # Exp heads the ACT critical chain; psum holds
                            # 1024*z so fold the descale into the Exp scale
                            last_exp = nc.scalar.activation(
                                E[:, dt_, :], psz[:, :], AF.Exp, scale=-Z_DESCALE
                            )
                            nc.vector.tensor_copy(G[:, 2 + dt_, :], psz[:, :])
                    # softplus(-z) = ln(exp(-z)+1); gate Lns on the group's
                    # last Exp to batch ACT table sets
                    for t in group:
                        for dt_ in range(2):
                            ln = nc.scalar.activation(
                                group_G[t][:, dt_, :],
                                group_E[t][:, dt_, :],
                                AF.Ln,
                                bias=1.0,
                            )
                            add_dep_helper(
                                ln.ins, last_exp.ins, sync=False,
                                reason="batch ACT Ln block after Exp block",
                            )
                    if not started:
                        # deferred weight loads: keep the startup DMA queues
                        # clear for xt/wd0 (critical path)
                        dma = nc.sync.dma_start(
                            smat_sb[:, :, :],
                            smat[:, :].rearrange("(k p) q -> p k q", p=128),
                        )
                        for t in range(n_trees):
                            nc.sync.dma_start(
                                w2_sb[:, t, :, :],
                                w2[t, :, :].rearrange("(l p) c -> p l c", p=128),
                            )
                        started = True
                    for t in group:
                        for lt in range(2):
                            plp = plpp.tile([128, CHUNK], F32, tag="plp")
                            for k in range(4):
                                nc.tensor.matmul(
                                    plp[:, :],
                                    smat_sb[:, k, lt * 128 : (lt + 1) * 128],
                                    group_G[t][:, k, :],
                                    start=(k == 0),
                                    stop=(k == 3),
                                )
                            nc.scalar.activation(
                                lpT[:, t, lt, :], plp[:, :], AF.Exp, scale=-1.0
                            )
                emit_mm2(ci, lpT)
    nc.compile()
    return nc


def _smat_np():
    # rows 0:256 multiply softplus(-z) (path indicator); rows 256:512 multiply
    # the raw psum (1024*z), so carry the 1/1024 descale here.
    S = np.zeros((512, NQ), np.float32)
    q = np.arange(NQ)
    for n in range(8):
        node = (2**n - 1) + (q >> (8 - n))
        branch = (q >> (7 - n)) & 1
        S[node, q] += 1.0
        S[256 + node, q] += branch.astype(np.float32) * np.float32(Z_DESCALE)
    return S


def _prep_weights(w_d, w_l, n_trees=N_TREES):
    bf16 = ml_dtypes.bfloat16
    fp8 = ml_dtypes.float8_e4m3
    w_l = np.asarray(w_l, dtype=np.float32)
    m = w_l.max(axis=-1, keepdims=True)
    e = np.exp(w_l - m, dtype=np.float32)
    sm = e / e.sum(axis=-1, keepdims=True)
    w2 = ((sm[:, 0::2, :] + sm[:, 1::2, :]) * np.float32(1.0 / n_trees)).astype(bf16)
    wd_p = np.zeros((n_trees, IN_DIM, ND_PAD), np.float32)
    wd_p[:, :, : w_d.shape[2]] = w_d * np.float32(WD_SCALE)
    return wd_p.astype(fp8), _smat_np(), w2


last_bass_results = None


def kernel(x, w_d, w_l):
    global last_bass_results
    x = np.asarray(x)
    wd_8, S, w2 = _prep_weights(np.asarray(w_d), np.asarray(w_l))
    x_8 = (x * np.float32(X_SCALE)).astype(ml_dtypes.float8_e4m3)
    in_maps = []
    for c in range(N_CORES):
        xtc = np.ascontiguousarray(x_8[c * B_LOC : (c + 1) * B_LOC, :].T)
        in_maps.append({"xt": xtc, "wd": wd_8, "smat": S, "w2": w2})
    if "nc" not in _CACHE:
        _CACHE["nc"] = _build()
    res = run_bass_kernel_spmd(_CACHE["nc"], in_maps, core_ids=list(range(N_CORES)))
    last_bass_results = res
    return np.concatenate([res.results[c]["out"] for c in range(N_CORES)], axis=0)
